# revision 1
# baseline (speedup 1.0000x reference)
"""Multi-head self-attention (B=2, S=4096, D=512, H=8, Dh=64) on 8 TRN2 cores.

Sharding: core i handles batch b = i//4 and head-pair hp = i%4 (heads 2*hp,
2*hp+1).  Each core computes Q/K/V projections for its two heads, flash-style
attention (no-max softmax; scores range is +-9 so exp is safe), and a partial
out-projection.  Host sums the 4 partial outputs per batch and transposes back.

All device tensors use transposed layouts (feature dim on partitions) so every
matmul has its contraction dim on the partition axis with no on-device
transposes:
  xt  [512, S]  = X[b].T
  wq/wk/wv [512, 128] = W[:, hp*128:(hp+1)*128]
  wo  [128, 512] = Wo[hp*128:(hp+1)*128, :]
  yt  [512, S]  = partial (Y[b]).T

Matmuls run as float32r (single-pass PE mode, 1 cycle/row vs 4 for fp32).

TRN2 quirk: an fp32/f32r matmul self-loads weights and its S3_LW slot encodes
exactly ONE sync wait; walrus cannot legalize more on a Matmult ("Too many
sync wait commands").  `_legalize_matmul_waits` post-processes the scheduled
module: extra waits move onto injected single-wait PE no-ops placed directly
before the matmul in its block — semantically identical, walrus-legal.
"""

import sys
from contextlib import ExitStack

for _p in ("/opt/trn_rl_repo",):
    if _p not in sys.path:
        sys.path.insert(0, _p)

import numpy as np

import concourse.bass as bass
import concourse.tile as tile
from concourse import mybir
from concourse.bass_utils import run_bass_kernel_spmd

F32 = mybir.dt.float32
F32R = mybir.dt.float32r
MM_DT = F32R     # single-pass PE mode: 1 cycle/row vs 4 for full fp32
D = 512          # model dim
DH = 64          # head dim
P = 128          # partitions
B = 2
H = 8
S_FULL = 4096
N_CORES = 8
NC_T = D // P    # 4 contraction tiles over model dim

LAST_RESULTS = None  # test harness reads exec_time_ns from here


def _emit(nc: bass.Bass, tc: "tile.TileContext", ctx: ExitStack, S: int):
    """Emit the per-core program. Parameterized by S for small-sim testing."""
    NS = S // 512            # 512-wide seq blocks
    NK = S // P              # 128-row key tiles
    QB = 1024 if S >= 1024 else S
    NQB = S // QB            # scores q-blocks
    QH = QB // 512           # 512-wide halves per q-block
    inv_scale = 1.0 / np.sqrt(DH)

    def mm(out, lhsT, rhs, start=True, stop=True):
        return nc.tensor.matmul(out, lhsT, rhs, start=start, stop=stop)

    xt = nc.declare_dram_parameter("xt", [D, S], MM_DT, isOutput=False)
    wq = nc.declare_dram_parameter("wq", [D, P], MM_DT, isOutput=False)
    wk = nc.declare_dram_parameter("wk", [D, P], MM_DT, isOutput=False)
    wv = nc.declare_dram_parameter("wv", [D, P], MM_DT, isOutput=False)
    wo = nc.declare_dram_parameter("wo", [P, D], MM_DT, isOutput=False)
    yt = nc.declare_dram_parameter("yt", [D, S], F32, isOutput=True)

    const = ctx.enter_context(tc.tile_pool(name="const", bufs=1))

    # ---- load inputs straight to SBUF ----
    xt_sb = []
    for c in range(NC_T):
        t = const.tile([P, S], MM_DT, tag=f"xt{c}", name=f"xt{c}")
        nc.sync.dma_start(out=t[:], in_=xt[c * P:(c + 1) * P, :])
        xt_sb.append(t)
    w_sb = {}
    for name, ap in (("wq", wq), ("wk", wk), ("wv", wv)):
        tiles = []
        for c in range(NC_T):
            t = const.tile([P, P], MM_DT, tag=f"{name}{c}", name=f"{name}{c}")
            nc.sync.dma_start(out=t[:], in_=ap[c * P:(c + 1) * P, :])
            tiles.append(t)
        w_sb[name] = tiles
    wo_sb = const.tile([P, D], MM_DT, tag="wo")
    nc.sync.dma_start(out=wo_sb[:], in_=wo[:, :])

    # persistent intermediates
    qt_sb = const.tile([P, S], MM_DT, tag="qt")      # [2*64 d, S] stacked heads
    kt_sb = const.tile([P, S], MM_DT, tag="kt")
    # V with a ones column appended per k-tile: [128 k, NK*65]; col 64 == 1.0
    vones = [const.tile([P, NK * (DH + 1)], MM_DT, tag=f"vones{h}", name=f"vones{h}")
             for h in range(2)]
    # column 64 of each 65-wide block must be 1.0; memset can't write f32r,
    # but a DVE copy can (it rounds on output)
    konst = const.tile([P, NK, 1], F32, tag="konst")
    nc.vector.memset(konst[:], 1.0)
    for h in range(2):
        vv = vones[h].rearrange("p (k c) -> p k c", c=DH + 1)
        nc.vector.tensor_copy(vv[:, :, DH:DH + 1], konst[:])
    ctx_sb = const.tile([P, S], MM_DT, tag="ctx")    # context^T, stacked heads

    # single PSUM pool: tag "s" (3 x [128,QB]) + tag "ctx" (1 x [65,QB]) = 8 banks
    ps = ctx.enter_context(tc.tile_pool(name="ps", bufs=3, space="PSUM"))
    es = ctx.enter_context(tc.tile_pool(name="es", bufs=3))
    bcp = ctx.enter_context(tc.tile_pool(name="bcp", bufs=2))
    rtp = ctx.enter_context(tc.tile_pool(name="rtp", bufs=2))
    rdp = ctx.enter_context(tc.tile_pool(name="rdp", bufs=2, space="DRAM"))

    # ---- phase A: V first, then Q/K ----
    for k in range(NK):
        ksl = slice(k * P, (k + 1) * P)
        pv = ps.tile([P, P], F32, tag="s", name="pv")
        for c in range(NC_T):
            mm(pv[:], xt_sb[c][:, ksl], w_sb["wv"][c][:],
               start=(c == 0), stop=(c == NC_T - 1))
        for h in range(2):
            nc.vector.tensor_copy(
                vones[h][:, k * (DH + 1):k * (DH + 1) + DH],
                pv[:, h * DH:(h + 1) * DH])
    for qb in range(NS):
        sl = slice(qb * 512, (qb + 1) * 512)
        pq = ps.tile([P, 512], F32, tag="s", name="pq")
        for c in range(NC_T):
            mm(pq[:], w_sb["wq"][c][:], xt_sb[c][:, sl],
               start=(c == 0), stop=(c == NC_T - 1))
        nc.vector.tensor_copy(qt_sb[:, sl], pq[:])
        pk = ps.tile([P, 512], F32, tag="s", name="pk")
        for c in range(NC_T):
            mm(pk[:], w_sb["wk"][c][:], xt_sb[c][:, sl],
               start=(c == 0), stop=(c == NC_T - 1))
        nc.vector.tensor_copy(kt_sb[:, sl], pk[:])

    # ---- phase B: attention (flash, no-max softmax), fused normalize ----
    for h in range(2):
        hsl = slice(h * DH, (h + 1) * DH)
        for qb in range(NQB):
            qsl = slice(qb * QB, (qb + 1) * QB)
            ctx_ps = ps.tile([DH + 1, QB], F32, tag="ctx", bufs=1, name="ctx_ps")
            for k in range(NK):
                s_ps = ps.tile([P, QB], F32, tag="s", name="s_ps")
                lhs_k = kt_sb[hsl, k * P:(k + 1) * P]
                for j in range(QH):
                    jsl = slice(qb * QB + j * 512, qb * QB + (j + 1) * 512)
                    mm(s_ps[:, j * 512:(j + 1) * 512], lhs_k, qt_sb[hsl, jsl])
                e_sb = es.tile([P, QB], MM_DT, tag="e", name="e_sb")
                nc.scalar.activation(e_sb[:], s_ps[:],
                                     mybir.ActivationFunctionType.Exp,
                                     scale=inv_scale)
                vo = vones[h][:, k * (DH + 1):(k + 1) * (DH + 1)]
                for j in range(QH):
                    mm(ctx_ps[:, j * 512:(j + 1) * 512], vo,
                       e_sb[:, j * 512:(j + 1) * 512],
                       start=(k == 0), stop=(k == NK - 1))
            # drain + normalize this block
            rt = rtp.tile([1, QB], F32, tag="rt", name="rt")
            nc.vector.tensor_copy(rt[0:1, :], ctx_ps[DH:DH + 1, :])
            nc.vector.reciprocal(rt[0:1, :], rt[0:1, :])
            bc = bcp.tile([DH, QB], F32, tag="bc", name="bc")
            # partition-broadcast rt row 0 to 64 partitions: bounce through
            # DRAM, whose APs allow a stride-0 partition dim (SBUF APs don't)
            rtd = rdp.tile([1, QB], F32, tag="rtd", name="rtd")
            nc.sync.dma_start(out=rtd[:], in_=rt[0:1, :])
            rtd_bcast = bass.AP(tensor=rtd.tensor, offset=rtd.offset,
                                ap=[[0, DH]] + list(rtd[0:1, :].ap)[1:])
            nc.sync.dma_start(out=bc[:], in_=rtd_bcast)
            nc.vector.tensor_mul(ctx_sb[hsl, qsl], ctx_ps[:DH, :], bc[:])

    # ---- phase C: out-projection (partial; host sums across cores) ----
    with tc.tile_pool(name="osb", bufs=2) as osb:
        for e in range(NC_T):
            for sb in range(NS):
                sl = slice(sb * 512, (sb + 1) * 512)
                o_ps = ps.tile([P, 512], F32, tag="s", name="o_ps")
                mm(o_ps[:], wo_sb[:, e * P:(e + 1) * P], ctx_sb[:, sl])
                o_sb = osb.tile([P, 512], F32, tag="osb", name="o_sb")
                nc.scalar.copy(o_sb[:], o_ps[:])
                nc.sync.dma_start(out=yt[e * P:(e + 1) * P, sl], in_=o_sb[:])


_TPB_ENGINES = {mybir.EngineType.PE, mybir.EngineType.Activation,
                mybir.EngineType.DVE, mybir.EngineType.Pool}


def _legalize_matmul_waits(nc: bass.Bass) -> int:
    """Walrus encodes only ONE sync wait on TPB compute instructions (seen on
    Matmult and TensorCopy).  Move extra waits onto injected same-engine
    no-ops (one wait each) placed immediately before the instruction in its
    block: same semantics, legal encoding."""
    n_fixed = 0
    for f in nc.m.functions:
        for bb in f.blocks:
            out = []
            changed = False
            for ins in bb.instructions:
                si = ins.sync_info
                if (getattr(ins, "engine", None) is not None
                        and si is not None and len(si.on_wait) > 1):
                    for idx, w in enumerate(si.on_wait[:-1]):
                        nop = mybir.InstNoOp(name=f"{ins.name}-lgw{idx}",
                                             ins=[], outs=[])
                        nop.engine = ins.engine
                        nop.sync_info = mybir.SyncInfo(on_wait=[w], on_update=[])
                        out.append(nop)
                    ins.sync_info = mybir.SyncInfo(on_wait=[si.on_wait[-1]],
                                                   on_update=si.on_update)
                    n_fixed += 1
                    changed = True
                out.append(ins)
            if changed:
                bb.instructions = out
    return n_fixed


def build(S: int = S_FULL, legalize: bool = False) -> bass.Bass:
    nc = bass.Bass()
    with ExitStack() as ctx:
        if MM_DT == F32R:
            ctx.enter_context(nc.allow_low_precision(
                reason="f32r matmul operands (11-bit mantissa)"))
        tc = ctx.enter_context(tile.TileContext(nc))
        _emit(nc, tc, ctx, S)
    if legalize:
        # only for the walrus/hardware path; CoreSim wants updates on every
        # instruction and doesn't enforce the 1-wait Matmult limit
        _legalize_matmul_waits(nc)
    return nc


_NC_CACHE = {}


def _get_nc(S: int) -> bass.Bass:
    if S not in _NC_CACHE:
        _NC_CACHE[S] = build(S, legalize=True)
    return _NC_CACHE[S]


def _round_f32r(a):
    if MM_DT != F32R:
        return np.ascontiguousarray(a, dtype=np.float32)
    u = np.ascontiguousarray(a, dtype=np.float32).view(np.uint32)
    r = (u + 0x7FF + ((u >> 12) & 1)) & np.uint32(0xFFFFF000)
    return r.view(np.float32)


def make_in_maps(X, Wq, Wk, Wv, Wo):
    xts = [_round_f32r(X[b].T) for b in range(B)]
    in_maps = []
    for i in range(N_CORES):
        b, hp = divmod(i, 4)  # 4 head-pairs per batch
        csl = slice(hp * P, (hp + 1) * P)
        in_maps.append({
            "xt": xts[b],
            "wq": _round_f32r(Wq[:, csl]),
            "wk": _round_f32r(Wk[:, csl]),
            "wv": _round_f32r(Wv[:, csl]),
            "wo": _round_f32r(Wo[csl, :]),
        })
    return in_maps


def kernel(X, Wq, Wk, Wv, Wo, _trace=False):
    global LAST_RESULTS
    X = np.asarray(X, dtype=np.float32)
    S = X.shape[1]
    nc = _get_nc(S)
    in_maps = make_in_maps(X, np.asarray(Wq, np.float32), np.asarray(Wk, np.float32),
                           np.asarray(Wv, np.float32), np.asarray(Wo, np.float32))
    res = run_bass_kernel_spmd(nc, in_maps, list(range(N_CORES)), trace=_trace)
    LAST_RESULTS = res
    Y = np.zeros((B, S, D), dtype=np.float32)
    for i in range(N_CORES):
        Y[i // 4] += res.results[i]["yt"].T
    return Y



# revision 5
# speedup vs baseline: 1.0667x; 1.0667x over previous
"""Multi-head self-attention (B=2, S=4096, D=512, H=8, Dh=64) on 8 TRN2 cores.

Sharding: core i handles batch b = i//4 and head-pair hp = i%4 (heads 2*hp,
2*hp+1).  Each core computes Q/K/V projections for its two heads, flash-style
attention (no-max softmax; scores range is +-9 so exp is safe), and a partial
out-projection.  Host sums the 4 partial outputs per batch and transposes back.

v2 (bf16 pipeline): all matmul operands are bfloat16.  bf16 matmuls use
separate LDWEIGHTS instructions that the PE's 64-deep reorder window pulls
into the background weight buffer (plus FWL), so weight loads hide under the
streaming passes -- unlike fp32r matmuls, which self-load weights serially
(~107ns per matmul).  The kernel is restructured so the ACT engine (exp,
1 elem/cycle/lane @1.2GHz = the softmax roofline, ~290us for 33.5M exps)
never stalls:
  - scores PSUM tiles double-buffered, ctx PSUM double-buffered (normalize of
    block i runs under block i+1's compute; no PE idle -> no HAM re-throttle)
  - K projection accumulates chunk-by-chunk as the X DMA lands
  - out-projection of q-block i runs on the spare ctx-ring slot inside block
    i+1, copies on DVE (never ACT), output DMA overlapped
  - softmax normalize via reciprocal_approx_fast + DRAM-bounce partition
    broadcast, all off the critical path

Layouts (feature dim on partitions; every matmul contracts on partitions):
  xt  [512, S]  = X[b].T                       (bf16)
  wq/wk/wv [512, 128] = W[:, hp*128:(hp+1)*128] (bf16)
  wo  [128, 512] = Wo[hp*128:(hp+1)*128, :]     (bf16)
  yt  [512, S]  = partial (Y[b]).T              (fp32)

TRN2 quirk: walrus legalizes only ONE sync wait on TPB compute instructions.
`_legalize_matmul_waits` moves extra waits onto injected single-wait no-ops.
"""

import sys
from contextlib import ExitStack

for _p in ("/opt/trn_rl_repo",):
    if _p not in sys.path:
        sys.path.insert(0, _p)

import numpy as np

import concourse.bass as bass
import concourse.tile as tile
from concourse import mybir
from concourse.bass_utils import run_bass_kernel_spmd

F32 = mybir.dt.float32
BF16 = mybir.dt.bfloat16
MM_DT = BF16
D = 512          # model dim
DH = 64          # head dim
P = 128          # partitions
B = 2
H = 8
S_FULL = 4096
N_CORES = 8
NC_T = D // P    # 4 contraction tiles over model dim

LAST_RESULTS = None  # test harness reads exec_time_ns from here


def _emit(nc: bass.Bass, tc: "tile.TileContext", ctx: ExitStack, S: int):
    """Emit the per-core program. Parameterized by S for small-sim testing."""
    NK = S // P              # 128-row key tiles
    QB = 1024 if S >= 1024 else S
    NQB = S // QB            # attention q-blocks
    NJ = QB // 512           # 512-col matmul passes per q-block
    inv_scale = 1.0 / np.sqrt(DH)

    def mm(out, lhsT, rhs, start=True, stop=True):
        return nc.tensor.matmul(out, lhsT, rhs, start=start, stop=stop)

    xt = nc.declare_dram_parameter("xt", [D, S], MM_DT, isOutput=False)
    wq = nc.declare_dram_parameter("wq", [D, P], MM_DT, isOutput=False)
    wk = nc.declare_dram_parameter("wk", [D, P], MM_DT, isOutput=False)
    wv = nc.declare_dram_parameter("wv", [D, P], MM_DT, isOutput=False)
    wo = nc.declare_dram_parameter("wo", [P, D], MM_DT, isOutput=False)
    yt = nc.declare_dram_parameter("yt", [D, S], F32, isOutput=True)

    const = ctx.enter_context(tc.tile_pool(name="const", bufs=1))

    # ---- weight DMA first (small), then X chunks (1MB each) ----
    w_sb = {}
    for name, ap in (("wq", wq), ("wk", wk), ("wv", wv)):
        tiles = []
        for c in range(NC_T):
            t = const.tile([P, P], MM_DT, tag=f"{name}{c}", name=f"{name}{c}")
            nc.sync.dma_start(out=t[:], in_=ap[c * P:(c + 1) * P, :])
            tiles.append(t)
        w_sb[name] = tiles
    wo_sb = const.tile([P, D], MM_DT, tag="wo")
    nc.sync.dma_start(out=wo_sb[:], in_=wo[:, :])
    xt_sb = []
    for c in range(NC_T):
        t = const.tile([P, S], MM_DT, tag=f"xt{c}", name=f"xt{c}")
        nc.sync.dma_start(out=t[:], in_=xt[c * P:(c + 1) * P, :])
        xt_sb.append(t)

    # persistent intermediates
    qt_sb = const.tile([P, S], MM_DT, tag="qt")      # [2*64 d, S] stacked heads
    kt_sb = const.tile([P, S], MM_DT, tag="kt")
    # V with a ones column appended per k-tile: [128 k, NK*65]; col 64 == 1.0
    vones = [const.tile([P, NK * (DH + 1)], MM_DT, tag=f"vones{h}", name=f"vones{h}")
             for h in range(2)]
    konst = const.tile([P, NK, 1], F32, tag="konst")
    nc.vector.memset(konst[:], 1.0)
    for h in range(2):
        vv = vones[h].rearrange("p (k c) -> p k c", c=DH + 1)
        nc.vector.tensor_copy(vv[:, :, DH:DH + 1], konst[:])
    ctx_sb = const.tile([P, S], MM_DT, tag="ctx")    # context^T, stacked heads

    # PSUM: tag "s" 2 x [128,1024] (4 banks) + tag "ctx" 2 x [65,1024] (4) = 8
    ps = ctx.enter_context(tc.tile_pool(name="ps", bufs=2, space="PSUM"))
    es = ctx.enter_context(tc.tile_pool(name="es", bufs=4))
    bcp = ctx.enter_context(tc.tile_pool(name="bcp", bufs=2))
    rtp = ctx.enter_context(tc.tile_pool(name="rtp", bufs=2))
    rdp = ctx.enter_context(tc.tile_pool(name="rdp", bufs=2, space="DRAM"))
    osb = ctx.enter_context(tc.tile_pool(name="osb", bufs=4))

    # ---- phase A ----
    # K projection: c-inner accumulation so chunk c's matmuls start as soon as
    # chunk c's DMA lands; K is DMA-bound, Q/V then run at PE speed.
    def proj_block(dst, wname, lo, hi):
        """dst[:, lo:hi] = (W^T x)[:, lo:hi] accumulated over the 4 chunks."""
        pq = ps.tile([P, hi - lo], F32, tag="s", name=f"p{wname}")
        for j in range(0, hi - lo, 512):
            w512 = min(512, hi - lo - j)
            for c in range(NC_T):
                mm(pq[:, j:j + w512], w_sb[wname][c][:],
                   xt_sb[c][:, lo + j:lo + j + w512],
                   start=(c == 0), stop=(c == NC_T - 1))
        nc.vector.tensor_copy(dst[:, lo:hi], pq[:])

    KBLK = min(1024, S)
    for bp in range(S // KBLK):
        proj_block(kt_sb, "wk", bp * KBLK, (bp + 1) * KBLK)

    # V projection: all k-tiles (pv on the "s" ring; [128,128] per tile)
    def proj_v(k):
        ksl = slice(k * P, (k + 1) * P)
        pv = ps.tile([P, P], F32, tag="s", name="pv")
        for c in range(NC_T):
            mm(pv[:], xt_sb[c][:, ksl], w_sb["wv"][c][:],
               start=(c == 0), stop=(c == NC_T - 1))
        for h in range(2):
            nc.vector.tensor_copy(
                vones[h][:, k * (DH + 1):k * (DH + 1) + DH],
                pv[:, h * DH:(h + 1) * DH])

    for k in range(NK):
        proj_v(k)

    # Q projection (all blocks)
    for bp in range(S // KBLK):
        proj_block(qt_sb, "wq", bp * KBLK, (bp + 1) * KBLK)

    # ---- phase B + C interleaved ----
    def attn_block(h, qb):
        hsl = slice(h * DH, (h + 1) * DH)
        ctx_ps = ps.tile([DH + 1, QB], F32, tag="ctx", name="ctx_ps")
        for k in range(NK):
            s_ps = ps.tile([P, QB], F32, tag="s", name="s_ps")
            lhs_k = kt_sb[hsl, k * P:(k + 1) * P]
            for j in range(NJ):
                jsl = slice(qb * QB + j * 512, qb * QB + (j + 1) * 512)
                mm(s_ps[:, j * 512:(j + 1) * 512], lhs_k, qt_sb[hsl, jsl])
            e_sb = es.tile([P, QB], MM_DT, tag="e", name="e_sb")
            nc.scalar.activation(e_sb[:], s_ps[:],
                                 mybir.ActivationFunctionType.Exp,
                                 scale=inv_scale)
            vo = vones[h][:, k * (DH + 1):(k + 1) * (DH + 1)]
            for j in range(NJ):
                mm(ctx_ps[:, j * 512:(j + 1) * 512], vo,
                   e_sb[:, j * 512:(j + 1) * 512],
                   start=(k == 0), stop=(k == NK - 1))
        # normalize this block (pipelines under the next block's compute)
        qsl = slice(qb * QB, (qb + 1) * QB)
        rinv = rtp.tile([1, QB], F32, tag="rt", name="rinv")
        nc.vector.reciprocal(rinv[0:1, :], ctx_ps[DH:DH + 1, :])
        # partition-broadcast rinv row 0 to 64 partitions: bounce through
        # DRAM, whose APs allow a stride-0 partition dim (SBUF APs don't)
        rtd = rdp.tile([1, QB], F32, tag="rtd", name="rtd")
        nc.sync.dma_start(out=rtd[:], in_=rinv[0:1, :])
        rtd_bcast = bass.AP(tensor=rtd.tensor, offset=rtd.offset,
                            ap=[[0, DH]] + list(rtd[0:1, :].ap)[1:])
        bc = bcp.tile([DH, QB], F32, tag="bc", name="bc")
        nc.sync.dma_start(out=bc[:], in_=rtd_bcast)
        nc.vector.tensor_mul(ctx_sb[hsl, qsl], ctx_ps[:DH, :], bc[:])

    def out_proj(qb):
        """Partial out-projection for q-block qb (both heads' ctx ready).
        8 "s"-ring allocations (even count keeps the scores' double-buffer
        alternation intact); copies on DVE, never ACT."""
        for e in range(NC_T):
            for j in range(NJ):
                sl = slice(qb * QB + j * 512, qb * QB + (j + 1) * 512)
                o_ps = ps.tile([P, 512], F32, tag="s", name="o_ps")
                mm(o_ps[:], wo_sb[:, e * P:(e + 1) * P], ctx_sb[:, sl])
                o_sb = osb.tile([P, 512], F32, tag="osb", name="o_sb")
                nc.vector.tensor_copy(o_sb[:], o_ps[:])
                nc.sync.dma_start(out=yt[e * P:(e + 1) * P, sl], in_=o_sb[:])

    for qb in range(NQB):
        attn_block(0, qb)
        if qb > 0:
            # previous q-block's out-projection, emitted between this
            # q-block's two head-blocks (ctx_sb(qb-1) long since ready)
            out_proj(qb - 1)
        attn_block(1, qb)
    out_proj(NQB - 1)


_TPB_ENGINES = {mybir.EngineType.PE, mybir.EngineType.Activation,
                mybir.EngineType.DVE, mybir.EngineType.Pool}


def _legalize_matmul_waits(nc: bass.Bass) -> int:
    """Walrus encodes only ONE sync wait on TPB compute instructions (seen on
    Matmult and TensorCopy).  Move extra waits onto injected same-engine
    no-ops (one wait each) placed immediately before the instruction in its
    block: same semantics, legal encoding."""
    n_fixed = 0
    for f in nc.m.functions:
        for bb in f.blocks:
            out = []
            changed = False
            for ins in bb.instructions:
                si = ins.sync_info
                if (getattr(ins, "engine", None) is not None
                        and si is not None and len(si.on_wait) > 1):
                    for idx, w in enumerate(si.on_wait[:-1]):
                        nop = mybir.InstNoOp(name=f"{ins.name}-lgw{idx}",
                                             ins=[], outs=[])
                        nop.engine = ins.engine
                        nop.sync_info = mybir.SyncInfo(on_wait=[w], on_update=[])
                        out.append(nop)
                    ins.sync_info = mybir.SyncInfo(on_wait=[si.on_wait[-1]],
                                                   on_update=si.on_update)
                    n_fixed += 1
                    changed = True
                out.append(ins)
            if changed:
                bb.instructions = out
    return n_fixed


def build(S: int = S_FULL, legalize: bool = False) -> bass.Bass:
    nc = bass.Bass()
    with ExitStack() as ctx:
        ctx.enter_context(nc.allow_low_precision(
            reason="bf16 matmul operands / intermediates"))
        tc = ctx.enter_context(tile.TileContext(nc))
        _emit(nc, tc, ctx, S)
    if legalize:
        # only for the walrus/hardware path; CoreSim wants updates on every
        # instruction and doesn't enforce the 1-wait Matmult limit
        _legalize_matmul_waits(nc)
    return nc


_NC_CACHE = {}


def _get_nc(S: int) -> bass.Bass:
    if S not in _NC_CACHE:
        _NC_CACHE[S] = build(S, legalize=True)
    return _NC_CACHE[S]


def _bf16(a):
    import ml_dtypes
    return np.ascontiguousarray(np.asarray(a, dtype=np.float32)).astype(
        ml_dtypes.bfloat16)


def make_in_maps(X, Wq, Wk, Wv, Wo):
    X = np.asarray(X, dtype=np.float32)
    xts = [_bf16(X[b].T) for b in range(B)]
    in_maps = []
    for i in range(N_CORES):
        b, hp = divmod(i, 4)  # 4 head-pairs per batch
        csl = slice(hp * P, (hp + 1) * P)
        in_maps.append({
            "xt": xts[b],
            "wq": _bf16(Wq[:, csl]),
            "wk": _bf16(Wk[:, csl]),
            "wv": _bf16(Wv[:, csl]),
            "wo": _bf16(Wo[csl, :]),
        })
    return in_maps


def kernel(X, Wq, Wk, Wv, Wo, _trace=False):
    global LAST_RESULTS
    X = np.asarray(X, dtype=np.float32)
    S = X.shape[1]
    nc = _get_nc(S)
    in_maps = make_in_maps(X, np.asarray(Wq, np.float32), np.asarray(Wk, np.float32),
                           np.asarray(Wv, np.float32), np.asarray(Wo, np.float32))
    res = run_bass_kernel_spmd(nc, in_maps, list(range(N_CORES)), trace=_trace)
    LAST_RESULTS = res
    Y = np.zeros((B, S, D), dtype=np.float32)
    for i in range(N_CORES):
        Y[i // 4] += res.results[i]["yt"].T
    return Y


# revision 7
# speedup vs baseline: 1.4451x; 1.3548x over previous
"""Multi-head self-attention (B=2, S=4096, D=512, H=8, Dh=64) on 8 TRN2 cores.

Sharding: core i handles batch b = i//4 and head-pair hp = i%4 (heads 2*hp,
2*hp+1).  Each core computes Q/K/V projections for its two heads, flash-style
attention (no-max softmax; scores range is +-9 so exp is safe), and a partial
out-projection.  Host sums the 4 partial outputs per batch and transposes back.

v2 (bf16 pipeline): all matmul operands are bfloat16.  bf16 matmuls use
separate LDWEIGHTS instructions that the PE's 64-deep reorder window pulls
into the background weight buffer (plus FWL), so weight loads hide under the
streaming passes -- unlike fp32r matmuls, which self-load weights serially
(~107ns per matmul).  The kernel is restructured so the ACT engine (exp,
1 elem/cycle/lane @1.2GHz = the softmax roofline, ~290us for 33.5M exps)
never stalls:
  - scores PSUM tiles double-buffered, ctx PSUM double-buffered (normalize of
    block i runs under block i+1's compute; no PE idle -> no HAM re-throttle)
  - K projection accumulates chunk-by-chunk as the X DMA lands
  - out-projection of q-block i runs on the spare ctx-ring slot inside block
    i+1, copies on DVE (never ACT), output DMA overlapped
  - softmax normalize via reciprocal_approx_fast + DRAM-bounce partition
    broadcast, all off the critical path

Layouts (feature dim on partitions; every matmul contracts on partitions):
  xt  [512, S]  = X[b].T                       (bf16)
  wq/wk/wv [512, 128] = W[:, hp*128:(hp+1)*128] (bf16)
  wo  [128, 512] = Wo[hp*128:(hp+1)*128, :]     (bf16)
  yt  [512, S]  = partial (Y[b]).T              (fp32)

TRN2 quirk: walrus legalizes only ONE sync wait on TPB compute instructions.
`_legalize_matmul_waits` moves extra waits onto injected single-wait no-ops.
"""

import sys
from contextlib import ExitStack

for _p in ("/opt/trn_rl_repo",):
    if _p not in sys.path:
        sys.path.insert(0, _p)

import numpy as np

import concourse.bass as bass
import concourse.tile as tile
from concourse import mybir
from concourse.bass_utils import run_bass_kernel_spmd

F32 = mybir.dt.float32
BF16 = mybir.dt.bfloat16
MM_DT = BF16
D = 512          # model dim
DH = 64          # head dim
P = 128          # partitions
B = 2
H = 8
S_FULL = 4096
N_CORES = 8
NC_T = D // P    # 4 contraction tiles over model dim

LAST_RESULTS = None  # test harness reads exec_time_ns from here


def _emit(nc: bass.Bass, tc: "tile.TileContext", ctx: ExitStack, S: int):
    """Emit the per-core program. Parameterized by S for small-sim testing."""
    NK = S // P              # 128-row key tiles
    QB = 512                 # q-block (both heads processed per block)
    NQB = S // QB            # attention q-blocks
    inv_scale = 1.0 / np.sqrt(DH)

    def mm(out, lhsT, rhs, start=True, stop=True):
        return nc.tensor.matmul(out, lhsT, rhs, start=start, stop=stop)

    xt = nc.declare_dram_parameter("xt", [D, S], MM_DT, isOutput=False)
    wq = nc.declare_dram_parameter("wq", [D, P], MM_DT, isOutput=False)
    wk = nc.declare_dram_parameter("wk", [D, P], MM_DT, isOutput=False)
    wv = nc.declare_dram_parameter("wv", [D, P], MM_DT, isOutput=False)
    wo = nc.declare_dram_parameter("wo", [P, D], MM_DT, isOutput=False)
    yt = nc.declare_dram_parameter("yt", [D, S], F32, isOutput=True)

    const = ctx.enter_context(tc.tile_pool(name="const", bufs=1))

    # ---- weight DMA first (small), then X chunks (1MB each) ----
    w_sb = {}
    for name, ap in (("wq", wq), ("wk", wk), ("wv", wv)):
        tiles = []
        for c in range(NC_T):
            t = const.tile([P, P], MM_DT, tag=f"{name}{c}", name=f"{name}{c}")
            nc.sync.dma_start(out=t[:], in_=ap[c * P:(c + 1) * P, :])
            tiles.append(t)
        w_sb[name] = tiles
    wo_sb = const.tile([P, D], MM_DT, tag="wo")
    nc.sync.dma_start(out=wo_sb[:], in_=wo[:, :])
    xt_sb = []
    for c in range(NC_T):
        t = const.tile([P, S], MM_DT, tag=f"xt{c}", name=f"xt{c}")
        nc.sync.dma_start(out=t[:], in_=xt[c * P:(c + 1) * P, :])
        xt_sb.append(t)

    # persistent intermediates
    qt_sb = const.tile([P, S], MM_DT, tag="qt")      # [2*64 d, S] stacked heads
    kt_sb = const.tile([P, S], MM_DT, tag="kt")
    # V with a ones column appended per k-tile: [128 k, NK*65]; col 64 == 1.0
    vones = [const.tile([P, NK * (DH + 1)], MM_DT, tag=f"vones{h}", name=f"vones{h}")
             for h in range(2)]
    konst = const.tile([P, NK, 1], F32, tag="konst")
    nc.vector.memset(konst[:], 1.0)
    for h in range(2):
        vv = vones[h].rearrange("p (k c) -> p k c", c=DH + 1)
        nc.vector.tensor_copy(vv[:, :, DH:DH + 1], konst[:])
    ctx_sb = const.tile([P, S], MM_DT, tag="ctx")    # context^T, stacked heads

    # PSUM: tag "s" 2 x [128,1024] (4 banks) + tag "ctx" 2 x [65,1024] (4) = 8
    ps = ctx.enter_context(tc.tile_pool(name="ps", bufs=2, space="PSUM"))
    es = ctx.enter_context(tc.tile_pool(name="es", bufs=4))
    bcp = ctx.enter_context(tc.tile_pool(name="bcp", bufs=2))
    rtp = ctx.enter_context(tc.tile_pool(name="rtp", bufs=2))
    rdp = ctx.enter_context(tc.tile_pool(name="rdp", bufs=2, space="DRAM"))
    osb = ctx.enter_context(tc.tile_pool(name="osb", bufs=4))

    # ---- phase A ----
    # K projection: c-inner accumulation so chunk c's matmuls start as soon as
    # chunk c's DMA lands; K is DMA-bound, Q/V then run at PE speed.
    def proj_block(dst, wname, lo, hi):
        """dst[:, lo:hi] = (W^T x)[:, lo:hi] accumulated over the 4 chunks."""
        pq = ps.tile([P, hi - lo], F32, tag="s", name=f"p{wname}")
        for j in range(0, hi - lo, 512):
            w512 = min(512, hi - lo - j)
            for c in range(NC_T):
                mm(pq[:, j:j + w512], w_sb[wname][c][:],
                   xt_sb[c][:, lo + j:lo + j + w512],
                   start=(c == 0), stop=(c == NC_T - 1))
        nc.vector.tensor_copy(dst[:, lo:hi], pq[:])

    KBLK = min(1024, S)
    for bp in range(S // KBLK):
        proj_block(kt_sb, "wk", bp * KBLK, (bp + 1) * KBLK)

    # V projection: all k-tiles (pv on the "s" ring; [128,128] per tile)
    def proj_v(k):
        ksl = slice(k * P, (k + 1) * P)
        pv = ps.tile([P, P], F32, tag="s", name="pv")
        for c in range(NC_T):
            mm(pv[:], xt_sb[c][:, ksl], w_sb["wv"][c][:],
               start=(c == 0), stop=(c == NC_T - 1))
        for h in range(2):
            nc.vector.tensor_copy(
                vones[h][:, k * (DH + 1):k * (DH + 1) + DH],
                pv[:, h * DH:(h + 1) * DH])

    for k in range(NK):
        proj_v(k)

    # Q projection (all blocks)
    for bp in range(S // KBLK):
        proj_block(qt_sb, "wq", bp * KBLK, (bp + 1) * KBLK)

    # ---- phase B + C interleaved ----
    # Per (qb, k): the two heads' score matmuls are row-packed -- h0 uses PE
    # rows 0-63 (base_partition 0), h1 rows 64-127 (base_partition 64) -- and
    # run CONCURRENTLY in disjoint row-groups, writing the two 512-col halves
    # (= two different banks) of one [128,1024] PSUM tile.  A single N=1024
    # exp then covers both heads, keeping the ACT cadence at (1024+352)/1.2
    # ~= 1147ns per k-tile while PE streaming is only ~650ns (fits under the
    # ACT cadence even when the power manager halves the PE clock).
    def emit_out_tile(qb, idx):
        """One out-projection column tile for q-block qb on the "s" ring."""
        sl = slice(qb * QB, (qb + 1) * QB)
        o_ps = ps.tile([P, QB], F32, tag="s", name="o_ps")
        mm(o_ps[:, :QB], wo_sb[:, idx * P:(idx + 1) * P], ctx_sb[:, sl])
        o_sb = osb.tile([P, QB], F32, tag="osb", name="o_sb")
        nc.vector.tensor_copy(o_sb[:], o_ps[:, :QB])
        nc.sync.dma_start(out=yt[idx * P:(idx + 1) * P, sl], in_=o_sb[:])

    def normalize(h, qb, ctx_ps):
        """Emit rowsum reciprocal + partition-broadcast for one head-block.
        The [1,QB] rowsum row is reshaped to [64, QB//64] via a DRAM bounce
        so the DVE reciprocal costs ~QB//64 columns, not QB.  Returns the
        final tensor_mul emission (deferred by the caller to keep the DVE
        FIFO clear of long waits)."""
        hsl = slice(h * DH, (h + 1) * DH)
        qsl = slice(qb * QB, (qb + 1) * QB)
        NW = QB // DH        # columns per partition in the [64, NW] reshape
        rt = rtp.tile([1, QB], F32, tag="rt", name="rt")
        nc.vector.tensor_copy(rt[0:1, :], ctx_ps[DH:DH + 1, :])
        rd1 = rdp.tile([1, QB], F32, tag="rd1", name="rd1")
        nc.sync.dma_start(out=rd1[:], in_=rt[0:1, :])
        el = list(rd1[0:1, :].ap)[1]           # [elem_stride, QB]
        rd1_64 = bass.AP(tensor=rd1.tensor, offset=rd1.offset,
                         ap=[[el[0] * NW, DH], [el[0], NW]])
        r64 = rtp.tile([DH, NW], F32, tag="r64", name="r64")
        nc.sync.dma_start(out=r64[:], in_=rd1_64)
        rinv64 = rtp.tile([DH, NW], F32, tag="rinv64", name="rinv64")
        nc.vector.reciprocal(rinv64[:], r64[:])
        rd2 = rdp.tile([1, QB], F32, tag="rd2", name="rd2")
        el2 = list(rd2[0:1, :].ap)[1]
        rd2_64 = bass.AP(tensor=rd2.tensor, offset=rd2.offset,
                         ap=[[el2[0] * NW, DH], [el2[0], NW]])
        nc.sync.dma_start(out=rd2_64, in_=rinv64[:])
        rd2_bcast = bass.AP(tensor=rd2.tensor, offset=rd2.offset,
                            ap=[[0, DH], el2])
        bc = bcp.tile([DH, QB], F32, tag="bc", name="bc")
        nc.sync.dma_start(out=bc[:], in_=rd2_bcast)
        return lambda: nc.vector.tensor_mul(ctx_sb[hsl, qsl],
                                            ctx_ps[:DH, :], bc[:])

    for qb in range(NQB):
        qsl = slice(qb * QB, (qb + 1) * QB)
        ctx_h = [ps.tile([DH + 1, QB], F32, tag="ctx", bufs=4,
                         name=f"ctx_ps{h}") for h in range(2)]
        for k in range(NK):
            s_pair = ps.tile([P, 2 * QB], F32, tag="s", name="s_pair")
            for h in range(2):
                hsl = slice(h * DH, (h + 1) * DH)
                mm(s_pair[:, h * QB:(h + 1) * QB],
                   kt_sb[hsl, k * P:(k + 1) * P], qt_sb[hsl, qsl])
            e_pair = es.tile([P, 2 * QB], MM_DT, tag="e", name="e_pair")
            nc.scalar.activation(e_pair[:], s_pair[:],
                                 mybir.ActivationFunctionType.Exp,
                                 scale=inv_scale)
            for h in range(2):
                vo = vones[h][:, k * (DH + 1):(k + 1) * (DH + 1)]
                mm(ctx_h[h][:], vo, e_pair[:, h * QB:(h + 1) * QB],
                   start=(k == 0), stop=(k == NK - 1))
            # previous q-block's out-projection rides the "s" ring in pairs
            # (even allocation count keeps the double-buffer alternation);
            # at k=8 its ctx_sb inputs (norm muls ~4us after the boundary)
            # are safely ready
            if qb > 0 and k in (8, 9):
                emit_out_tile(qb - 1, 2 * (k - 8))
                emit_out_tile(qb - 1, 2 * (k - 8) + 1)
        # normalize both heads: long-latency DMA chains first, muls last so
        # the in-order DVE queue never parks on a DMA wait ahead of the
        # out-projection copies
        muls = [normalize(h, qb, ctx_h[h]) for h in range(2)]
        for m in muls:
            m()
    for idx in range(NC_T):
        emit_out_tile(NQB - 1, idx)


_TPB_ENGINES = {mybir.EngineType.PE, mybir.EngineType.Activation,
                mybir.EngineType.DVE, mybir.EngineType.Pool}


def _legalize_matmul_waits(nc: bass.Bass) -> int:
    """Walrus encodes only ONE sync wait on TPB compute instructions (seen on
    Matmult and TensorCopy).  Move extra waits onto injected same-engine
    no-ops (one wait each) placed immediately before the instruction in its
    block: same semantics, legal encoding."""
    n_fixed = 0
    for f in nc.m.functions:
        for bb in f.blocks:
            out = []
            changed = False
            for ins in bb.instructions:
                si = ins.sync_info
                if (getattr(ins, "engine", None) is not None
                        and si is not None and len(si.on_wait) > 1):
                    for idx, w in enumerate(si.on_wait[:-1]):
                        nop = mybir.InstNoOp(name=f"{ins.name}-lgw{idx}",
                                             ins=[], outs=[])
                        nop.engine = ins.engine
                        nop.sync_info = mybir.SyncInfo(on_wait=[w], on_update=[])
                        out.append(nop)
                    ins.sync_info = mybir.SyncInfo(on_wait=[si.on_wait[-1]],
                                                   on_update=si.on_update)
                    n_fixed += 1
                    changed = True
                out.append(ins)
            if changed:
                bb.instructions = out
    return n_fixed


def build(S: int = S_FULL, legalize: bool = False) -> bass.Bass:
    nc = bass.Bass()
    with ExitStack() as ctx:
        ctx.enter_context(nc.allow_low_precision(
            reason="bf16 matmul operands / intermediates"))
        tc = ctx.enter_context(tile.TileContext(nc))
        _emit(nc, tc, ctx, S)
    if legalize:
        # only for the walrus/hardware path; CoreSim wants updates on every
        # instruction and doesn't enforce the 1-wait Matmult limit
        _legalize_matmul_waits(nc)
    return nc


_NC_CACHE = {}


def _get_nc(S: int) -> bass.Bass:
    if S not in _NC_CACHE:
        _NC_CACHE[S] = build(S, legalize=True)
    return _NC_CACHE[S]


def _bf16(a):
    import ml_dtypes
    return np.ascontiguousarray(np.asarray(a, dtype=np.float32)).astype(
        ml_dtypes.bfloat16)


def make_in_maps(X, Wq, Wk, Wv, Wo):
    X = np.asarray(X, dtype=np.float32)
    xts = [_bf16(X[b].T) for b in range(B)]
    in_maps = []
    for i in range(N_CORES):
        b, hp = divmod(i, 4)  # 4 head-pairs per batch
        csl = slice(hp * P, (hp + 1) * P)
        in_maps.append({
            "xt": xts[b],
            "wq": _bf16(Wq[:, csl]),
            "wk": _bf16(Wk[:, csl]),
            "wv": _bf16(Wv[:, csl]),
            "wo": _bf16(Wo[csl, :]),
        })
    return in_maps


def kernel(X, Wq, Wk, Wv, Wo, _trace=False):
    global LAST_RESULTS
    X = np.asarray(X, dtype=np.float32)
    S = X.shape[1]
    nc = _get_nc(S)
    in_maps = make_in_maps(X, np.asarray(Wq, np.float32), np.asarray(Wk, np.float32),
                           np.asarray(Wv, np.float32), np.asarray(Wo, np.float32))
    res = run_bass_kernel_spmd(nc, in_maps, list(range(N_CORES)), trace=_trace)
    LAST_RESULTS = res
    Y = np.zeros((B, S, D), dtype=np.float32)
    for i in range(N_CORES):
        Y[i // 4] += res.results[i]["yt"].T
    return Y
